# revision 38
# baseline (speedup 1.0000x reference)
"""Trainium2 Bass kernel for nn_BlockAttnResTransformerBlock (sparse_attention).

Computes, for V = stack([completed_blocks (n=4), partial_block]):
  two inter-block-attention + projection sublayers applied to partial_block.

Everything is row-local over the flattened (b, t) axis (8192 rows), so we
shard 1024 rows per NeuronCore (8 cores, pure SPMD, no collectives).

Math per row r (d = 2048):
  logits_i = (x_i . (q*w_res)) * rsqrt(mean(x_i^2) + eps)   for each block i
  alpha = softmax_i(logits)  ->  h = sum_i alpha_i x_i
  out_sub = (h * rsqrt(mean(h^2)+eps)) @ (proj * w_norm).T
  partial += out_sub      (twice, with the second sublayer's V including the
                           updated partial block)

Key kernel tricks:
  - softmax without max-subtraction (logits are O(+-5)); unnormalized
    exp-weighted sums; 1/Z and rsqrt folded into one per-row scalar.
  - rsqrt computed as exp(-0.5*ln(x)) on ACT (both funcs in one table set).
  - fused multiply+reduce dots via scalar_tensor_tensor(accum_out=...).
  - residual adds done on the TensorEngine by appending an identity matmul
    to the PSUM accumulation group.
  - activations/weights in bf16 (fp32 accumulation in PSUM / accum_out).
  - x^T for the projection matmul produced by xbar DMA-transpose (bf16).
"""

import os
import sys

for _p in ("/opt/trn_rl_repo", "/root/.axon_site/_ro/trn_rl_repo"):
    if os.path.isdir(_p) and _p not in sys.path:
        sys.path.insert(0, _p)

import numpy as np
import ml_dtypes


def _ensure_ntff_hook():
    """Provide antenv.axon_hooks (NTFF profiling) if the image lacks it."""
    try:
        import antenv.axon_hooks  # noqa: F401
        return
    except ImportError:
        pass
    try:
        import types
        import antenv
        if "/root/.axon_site" not in sys.path and os.path.isdir("/root/.axon_site"):
            sys.path.insert(0, "/root/.axon_site")
        from trn_agent_boot.trn_boot import _ntff_profile_via_ctypes
        so = "/opt/axon/libaxon_pjrt.so"
        hook = _ntff_profile_via_ctypes(so) if os.path.exists(so) else None
        mod = types.ModuleType("antenv.axon_hooks")
        state = {"hook": hook}
        mod.get_axon_ntff_profile_hook = lambda: state["hook"]
        mod.set_axon_ntff_profile_hook = lambda h: state.__setitem__("hook", h)
        sys.modules["antenv.axon_hooks"] = mod
        antenv.axon_hooks = mod
    except Exception:
        pass


_ensure_ntff_hook()

import concourse.bass as bass
import concourse.bacc as bacc
import concourse.tile as tile
import concourse.mybir as mybir
from concourse.bass import ts
from concourse.bass_utils import run_bass_kernel_spmd
from concourse.masks import make_identity

BF16 = mybir.dt.bfloat16
F32 = mybir.dt.float32
AF = mybir.ActivationFunctionType
ALU = mybir.AluOpType

N_CORES = 8
N_BLK = 4          # completed blocks
D = 2048
ROWS_TOTAL = 8192  # b*t = 4*2048
R = ROWS_TOTAL // N_CORES   # rows per core
P = 128            # partitions / rows per tile
NT = R // P        # tiles per core (8)
KC = D // P        # contraction chunks (16)
NJ = D // 512      # psum bank chunks (4)
EPS = 1e-6

_CACHED_NC = None


def _fast_rsqrt(nc, statpool, y, x, n, eng=None):
    """y = rsqrt(x) for positive x, [P, n] f32, no ACT tables needed.

    Quake-style magic-constant seed + 2 Newton steps (~5e-6 rel err).
    Runs on `eng` (default gpsimd — tiny ops, keeps DVE free)."""
    eng = eng or nc.gpsimd
    x = x[:, 0:n]
    y = y[:, 0:n]
    iv = statpool.tile([P, n], mybir.dt.int32, tag=f"rsq_i{n}")
    f = statpool.tile([P, n], F32, tag=f"rsq_f{n}")
    t = statpool.tile([P, n], F32, tag=f"rsq_t{n}")
    eng.tensor_copy(out=f, in_=x.bitcast(mybir.dt.int32))  # int -> float
    eng.tensor_scalar(out=f, in0=f, scalar1=-0.5,
                      scalar2=float(0x5F3759DF), op0=ALU.mult, op1=ALU.add)
    eng.tensor_copy(out=iv, in_=f)                         # float -> int
    eng.tensor_copy(out=y.bitcast(mybir.dt.int32), in_=iv)  # raw bits
    for _ in range(2):
        eng.tensor_mul(out=t, in0=y, in1=y)
        eng.tensor_mul(out=t, in0=t, in1=x)
        eng.tensor_scalar(out=t, in0=t, scalar1=-0.5, scalar2=1.5,
                          op0=ALU.mult, op1=ALU.add)
        eng.tensor_mul(out=y, in0=y, in1=t)


def _emit_sublayer(nc, tc, pools, *, c_dram, ct_dram, part_dram, qbc_sb,
                   w_sb, ident, ss_store, s2_store, out_dram, out_dtype,
                   first_phase):
    """Emit one sublayer (8 tiles): part_new = part + proj(attn(V, part)).

    Emission is software-pipelined with a 1-tile skew: tile t's back half
    (x^T transpose + matmuls + writeback) is emitted after tile t+1's front
    half, so the late-dependency x^T DMA never head-of-line blocks the next
    tile's early transposes on the sync HWDGE ring."""
    (cpool, ppool, junkpool, statpool, wsumpool, xpool, xtpool, opool,
     psumpool, ctpool, stgpool, qpsum, wtmppool) = pools
    NB1 = N_BLK + 1
    state = {}

    for t in range(min(1, NT)):
        _emit_loads(nc, pools, state, t, c_dram=c_dram, ct_dram=ct_dram,
                    part_dram=part_dram, first_phase=first_phase)
    for t in range(NT + 1):
        if t + 1 < NT:
            _emit_loads(nc, pools, state, t + 1, c_dram=c_dram,
                        ct_dram=ct_dram, part_dram=part_dram,
                        first_phase=first_phase)
        if t < NT:
            _emit_front(nc, pools, state, t, qbc_sb=qbc_sb,
                        ss_store=ss_store, s2_store=s2_store,
                        first_phase=first_phase)
        if t >= 1:
            _emit_back(nc, pools, state, t - 1, w_sb=w_sb, ident=ident,
                       out_dram=out_dram, out_dtype=out_dtype)


def _emit_loads(nc, pools, state, t, *, c_dram, ct_dram, part_dram,
                first_phase):
    (cpool, ppool, junkpool, statpool, wsumpool, xpool, xtpool, opool,
     psumpool, ctpool, stgpool, qpsum, wtmppool) = pools
    NB1 = N_BLK + 1
    rows = slice(t * P, (t + 1) * P)
    cpt = cpool.tile([P, NB1, D], BF16, tag="c")
    nc.gpsimd.dma_start(out=cpt[:, 0:N_BLK, :], in_=c_dram[rows, :, :])
    nc.gpsimd.dma_start(out=cpt[:, N_BLK, :], in_=part_dram[rows, :])
    if first_phase:
        cT = ctpool.tile([P, N_BLK, KC, P], BF16, tag="cT")
        nc.gpsimd.dma_start(out=cT, in_=ct_dram[t])
    else:
        cT = None
    state[("ld", t)] = (cpt, cT)


def _emit_front(nc, pools, state, t, *, qbc_sb,
                ss_store, s2_store, first_phase):
    (cpool, ppool, junkpool, statpool, wsumpool, xpool, xtpool, opool,
     psumpool, ctpool, stgpool, qpsum, wtmppool) = pools
    NB1 = N_BLK + 1
    if True:
        rows = slice(t * P, (t + 1) * P)
        cpt, cT = state.pop(("ld", t))
        ct = cpt  # [:, i, :] views
        pt = cpt[:, N_BLK, :]

        sps = qpsum.tile([2, NB1 * P], F32, tag="sps")

        # ---- q-dots on the TensorEngine ----------------------------------
        # phase A: one M=2 pass over transposed [C..., P] computes s1 AND s2
        # for all 5 blocks; s2 of the completed blocks is stashed for B.
        # phase B: only the updated partial block needs a fresh dot.
        if first_phase:
            # pre-transposed C came straight from DRAM (host layout prep);
            # only the partial block needs the xbar transpose on device
            ptT = xtpool.tile([P, KC, P], BF16, tag="xt")
            nc.sync.dma_start_transpose(out=ptT, in_=pt)
            for c in range(KC):
                nc.tensor.matmul(sps[0:2, 0:N_BLK * P], lhsT=qbc_sb[:, c, :],
                                 rhs=cT[:, :, c, :], start=(c == 0),
                                 stop=(c == KC - 1))
            for c in range(KC):
                nc.tensor.matmul(sps[0:2, N_BLK * P:NB1 * P],
                                 lhsT=qbc_sb[:, c, :],
                                 rhs=ptT[:, c, :], start=(c == 0),
                                 stop=(c == KC - 1))
            st_stage = stgpool.tile([16, NB1 * P], BF16, tag="st_stage")
            nc.vector.tensor_copy(out=st_stage[0:2, :],
                                  in_=sps[0:2, 0:NB1 * P])
            sT = statpool.tile([P, NB1, 16], BF16, tag="sT")
            nc.sync.dma_start_transpose(out=sT, in_=st_stage[:, :])
            nc.vector.tensor_copy(out=s2_store[:, t * N_BLK:(t + 1) * N_BLK],
                                  in_=sT[:, 0:N_BLK, 1])
        else:
            ptT = xtpool.tile([P, KC, P], BF16, tag="xt")
            nc.sync.dma_start_transpose(out=ptT, in_=pt)
            for c in range(KC):
                nc.tensor.matmul(sps[0:2, 0:P], lhsT=qbc_sb[:, c, :],
                                 rhs=ptT[:, c, :], start=(c == 0),
                                 stop=(c == KC - 1))
            st_stage = stgpool.tile([16, NB1 * P], BF16, tag="st_stage")
            nc.vector.tensor_copy(out=st_stage[0:2, 0:P], in_=sps[0:2, 0:P])
            sT = statpool.tile([P, 16], BF16, tag="sTp")
            nc.sync.dma_start_transpose(out=sT, in_=st_stage[:, 0:P])

        # ---- per-row stats ----------------------------------------------
        # ss (sum of squares) for the 4 completed blocks + partial
        ssall = statpool.tile([P, NB1], F32, tag="ssall")
        if first_phase:
            for i in range(N_BLK):
                junk = junkpool.tile([P, D], BF16, tag="junk")
                nc.scalar.activation(out=junk, in_=ct[:, i, :], func=AF.Square,
                                     accum_out=ssall[:, i:i + 1])
            # stash for phase B (C doesn't change)
            nc.vector.tensor_copy(out=ss_store[:, t * N_BLK:(t + 1) * N_BLK],
                                  in_=ssall[:, 0:N_BLK])
        else:
            nc.vector.tensor_copy(out=ssall[:, 0:N_BLK],
                                  in_=ss_store[:, t * N_BLK:(t + 1) * N_BLK])
        junk = junkpool.tile([P, D], BF16, tag="junk")
        nc.scalar.activation(out=junk, in_=pt, func=AF.Square,
                             accum_out=ssall[:, N_BLK:NB1])

        # logits l = s * rsqrt(ss/D + eps)
        m = statpool.tile([P, NB1], F32, tag="m")
        nc.vector.tensor_scalar(out=m, in0=ssall, scalar1=1.0 / D, scalar2=EPS,
                                op0=ALU.mult, op1=ALU.add)
        rms = statpool.tile([P, NB1], F32, tag="rms")
        _fast_rsqrt(nc, statpool, rms, m, NB1)
        lg = statpool.tile([P, NB1], F32, tag="lg")
        if first_phase:
            nc.vector.tensor_mul(out=lg, in0=sT[:, :, 0], in1=rms)
        else:
            nc.vector.tensor_mul(out=lg[:, 0:N_BLK],
                                 in0=s2_store[:, t * N_BLK:(t + 1) * N_BLK],
                                 in1=rms[:, 0:N_BLK])
            nc.vector.tensor_mul(out=lg[:, N_BLK:NB1], in0=sT[:, 1:2],
                                 in1=rms[:, N_BLK:NB1])
        ew = statpool.tile([P, N_BLK + 1], F32, tag="ew")
        nc.scalar.activation(out=ew, in_=lg, func=AF.Exp)
        zr = statpool.tile([P, 2], F32, tag="zr")
        nc.vector.reduce_sum(out=zr[:, 0:1], in_=ew, axis=mybir.AxisListType.X)
        nc.vector.reciprocal(out=zr[:, 1:2], in_=zr[:, 0:1])  # r = 1/Z

        # ---- unnormalized weighted sum u = sum_i E_i * V_i ---------------
        # tensor_scalar (bf16 4x) + tensor_add (bf16 2x) per block
        w_acc = wsumpool.tile([P, D], BF16, tag="wsum")
        nc.vector.tensor_scalar(out=w_acc, in0=ct[:, 0, :],
                                scalar1=ew[:, 0:1], scalar2=None, op0=ALU.mult)
        for i in range(1, N_BLK + 1):
            src = pt if i == N_BLK else ct[:, i, :]
            tmp = wtmppool.tile([P, D], BF16, tag="wtmp")
            nc.vector.tensor_scalar(out=tmp, in0=src, scalar1=ew[:, i:i + 1],
                                    scalar2=None, op0=ALU.mult)
            w_next = wsumpool.tile([P, D], BF16, tag="wsum")
            nc.vector.tensor_add(out=w_next, in0=tmp, in1=w_acc)
            w_acc = w_next
        u = w_acc

        # ---- norm scalar c = r * rsqrt(r^2*ssu/D + eps) ------------------
        ssu = statpool.tile([P, 4], F32, tag="ssu")
        junk = junkpool.tile([P, D], BF16, tag="junk")
        nc.scalar.activation(out=junk, in_=u, func=AF.Square,
                             accum_out=ssu[:, 0:1])
        nc.vector.tensor_mul(out=ssu[:, 1:2], in0=zr[:, 1:2], in1=zr[:, 1:2])
        nc.vector.tensor_scalar(out=ssu[:, 2:3], in0=ssu[:, 0:1],
                                scalar1=ssu[:, 1:2], scalar2=1.0 / D,
                                op0=ALU.mult, op1=ALU.mult)
        nc.vector.tensor_scalar(out=ssu[:, 2:3], in0=ssu[:, 2:3], scalar1=EPS,
                                scalar2=None, op0=ALU.add)
        rsu = statpool.tile([P, 1], F32, tag="rsu")
        _fast_rsqrt(nc, statpool, rsu, ssu[:, 2:3], 1)
        nc.vector.tensor_mul(out=ssu[:, 3:4], in0=rsu, in1=zr[:, 1:2])

        # c is applied at the output copy instead of scaling u (so u^T can
        # be transposed as soon as the weighted sum finishes); the residual
        # is added as P/c via the identity matmul.
        rc = statpool.tile([P, 1], F32, tag="rc")
        nc.vector.reciprocal(out=rc, in_=ssu[:, 3:4])
        ptc = xpool.tile([P, D], BF16, tag="x")
        nc.vector.tensor_scalar(out=ptc, in0=pt, scalar1=rc,
                                scalar2=None, op0=ALU.mult)
        state[t] = (pt, ptc, u, ssu)


def _emit_back(nc, pools, state, t, *, w_sb, ident, out_dram, out_dtype):
    (cpool, ppool, junkpool, statpool, wsumpool, xpool, xtpool, opool,
     psumpool, ctpool, stgpool, qpsum, wtmppool) = pools
    pt, ptc, u, ssu = state.pop(t)
    rows = slice(t * P, (t + 1) * P)

    ut = xtpool.tile([P, KC, P], BF16, tag="xt")
    nc.sync.dma_start_transpose(out=ut, in_=u)

    # ---- projection matmul (two 2-bank halves) + residual add ------------
    po = opool.tile([P, D], out_dtype, tag="po")
    for h in range(2):
        psh = psumpool.tile([P, 1024], F32, tag="mm")
        if h == 0:
            # keep-warm filler: dependency-free matmuls that run during the
            # inter-tile gap so the PE HAM clock stays at 2.4 GHz
            for _ in range(16):
                nc.tensor.matmul(psh[0:2, 0:512], lhsT=ident[:, 0:2],
                                 rhs=w_sb[0][:, 0:512], start=True, stop=True,
                                 skip_group_check=True)
        for k in range(KC):
            for j in range(2):
                nc.tensor.matmul(psh[:, ts(j, 512)], lhsT=ut[:, k, :],
                                 rhs=w_sb[k][:, ts(h * 2 + j, 512)],
                                 start=(k == 0), stop=False)
        for j in range(2):
            nc.tensor.matmul(psh[:, ts(j, 512)], lhsT=ident,
                             rhs=ptc[:, ts(h * 2 + j, 512)], start=False,
                             stop=True)
        # out = c * (u @ W + P/c)  -- c applied as the copy's scale
        nc.scalar.activation(out=po[:, ts(h, 1024)], in_=psh, func=AF.Copy,
                             scale=ssu[:, 3:4])
    nc.gpsimd.dma_start(out=out_dram[rows, :], in_=po)


def _build_nc():
    nc = bacc.Bacc("TRN2", target_bir_lowering=False, debug=False,
                   num_devices=N_CORES)

    c_in = nc.dram_tensor("c", [R, N_BLK, D], BF16, kind="ExternalInput")
    ct_in = nc.dram_tensor("ctr", [NT, P, N_BLK, KC, P], BF16,
                           kind="ExternalInput")
    p_in = nc.dram_tensor("p", [R, D], BF16, kind="ExternalInput")
    w1_in = nc.dram_tensor("w1t", [D, D], BF16, kind="ExternalInput")
    w2_in = nc.dram_tensor("w2t", [D, D], BF16, kind="ExternalInput")
    qbc_in = nc.dram_tensor("qbc", [P, KC, 2], BF16, kind="ExternalInput")
    o_out = nc.dram_tensor("o", [R, D], F32, kind="ExternalOutput")
    p1_mid = nc.dram_tensor("p1mid", [R, D], BF16)  # internal scratch

    with tile.TileContext(nc) as tc:
        with (
            tc.tile_pool(name="singles", bufs=1) as singles,
            tc.tile_pool(name="weights", bufs=1) as wpool,
            tc.tile_pool(name="cpool", bufs=2) as cpool,
            tc.tile_pool(name="ppool", bufs=2) as ppool,
            tc.tile_pool(name="junk", bufs=1) as junkpool,
            tc.tile_pool(name="stat", bufs=5) as statpool,
            tc.tile_pool(name="wsum", bufs=3) as wsumpool,
            tc.tile_pool(name="wtmp", bufs=2) as wtmppool,
            tc.tile_pool(name="xpool", bufs=2) as xpool,
            tc.tile_pool(name="xtpool", bufs=2) as xtpool,
            tc.tile_pool(name="opool", bufs=2) as opool,
            tc.tile_pool(name="psum", bufs=3, space="PSUM") as psumpool,
            tc.tile_pool(name="qpsum", bufs=1, space="PSUM") as qpsum,
            tc.tile_pool(name="ctpool", bufs=2) as ctpool,
            tc.tile_pool(name="stgpool", bufs=2) as stgpool,
        ):
            ident = singles.tile([P, P], BF16)
            make_identity(nc, ident)
            qbc = singles.tile([P, KC, 2], BF16)
            nc.sync.dma_start(out=qbc, in_=qbc_in[:, :, :])
            ss_store = singles.tile([P, NT * N_BLK], F32)
            s2_store = singles.tile([P, NT * N_BLK], BF16)

            w1_view = w1_in.ap().rearrange("(c q) j -> c q j", q=P)
            w2_view = w2_in.ap().rearrange("(c q) j -> c q j", q=P)

            pools = (cpool, ppool, junkpool, statpool, wsumpool, xpool,
                     xtpool, opool, psumpool, ctpool, stgpool, qpsum,
                     wtmppool)

            # phase A: sublayer 1 (attn): p1mid = p + attn_out
            w1_sb = []
            for k in range(KC):
                wk = wpool.tile([P, D], BF16, tag=f"w{k}")
                nc.sync.dma_start(out=wk, in_=w1_view[k])
                w1_sb.append(wk)
            _emit_sublayer(nc, tc, pools, c_dram=c_in.ap(),
                           ct_dram=ct_in.ap(), part_dram=p_in.ap(),
                           qbc_sb=qbc, w_sb=w1_sb, ident=ident,
                           ss_store=ss_store, s2_store=s2_store,
                           out_dram=p1_mid.ap(),
                           out_dtype=BF16, first_phase=True)

            # phase B: sublayer 2 (mlp): o = p1mid + mlp_out
            w2_sb = []
            for k in range(KC):
                wk = wpool.tile([P, D], BF16, tag=f"w{k}")
                nc.sync.dma_start(out=wk, in_=w2_view[k])
                w2_sb.append(wk)
            _emit_sublayer(nc, tc, pools, c_dram=c_in.ap(),
                           ct_dram=ct_in.ap(), part_dram=p1_mid.ap(),
                           qbc_sb=qbc, w_sb=w2_sb, ident=ident,
                           ss_store=ss_store, s2_store=s2_store,
                           out_dram=o_out.ap(),
                           out_dtype=F32, first_phase=False)

    nc.compile()
    return nc


def _get_nc():
    global _CACHED_NC
    if _CACHED_NC is None:
        _CACHED_NC = _build_nc()
    return _CACHED_NC


def kernel(completed_blocks, partial_block, attn_norm_w, attn_proj,
           mlp_norm_w, mlp_proj, attn_res_query, attn_res_norm_w,
           mlp_res_query, mlp_res_norm_w, layer_in_block=1, **_ignored):
    bf16 = ml_dtypes.bfloat16
    cb = np.asarray(completed_blocks, np.float32)
    pb = np.asarray(partial_block, np.float32)

    # [n, b, t, d] -> [rows, n, d]
    c_host = np.ascontiguousarray(
        np.moveaxis(cb.reshape(N_BLK, ROWS_TOTAL, D), 0, 1)).astype(bf16)
    # pre-transposed per-core layout for the PE q-dots:
    # ctr[t, p, i, c, r] = C[t*128+r, i, c*128+p]
    ctr_host = np.ascontiguousarray(
        c_host.reshape(ROWS_TOTAL // P, P, N_BLK, KC, P)
        .transpose(0, 4, 2, 3, 1))
    p_host = pb.reshape(ROWS_TOTAL, D).astype(bf16)

    # fold the post-attention norm gain into the projection, transpose to [k, j]
    w1t = np.ascontiguousarray(
        (np.asarray(attn_proj, np.float32)
         * np.asarray(attn_norm_w, np.float32)[None, :]).T).astype(bf16)
    w2t = np.ascontiguousarray(
        (np.asarray(mlp_proj, np.float32)
         * np.asarray(mlp_norm_w, np.float32)[None, :]).T).astype(bf16)

    # fold the K-norm gain into the query; pre-broadcast across partitions
    qb1 = np.broadcast_to(
        (np.asarray(attn_res_query, np.float32)
         * np.asarray(attn_res_norm_w, np.float32)).astype(bf16), (P, D)).copy()
    qb2 = np.broadcast_to(
        (np.asarray(mlp_res_query, np.float32)
         * np.asarray(mlp_res_norm_w, np.float32)).astype(bf16), (P, D)).copy()
    # chunked lhsT for the PE q-dots: qbc[p, c, i] = q_i[c*128 + p]
    qbc = np.ascontiguousarray(
        np.stack([qb1[0].reshape(KC, P).T, qb2[0].reshape(KC, P).T],
                 axis=-1))  # [P, KC, 2] bf16

    nc = _get_nc()
    in_maps = []
    for i in range(N_CORES):
        rows = slice(i * R, (i + 1) * R)
        in_maps.append({
            "c": np.ascontiguousarray(c_host[rows]),
            "ctr": np.ascontiguousarray(ctr_host[i * NT:(i + 1) * NT]),
            "p": np.ascontiguousarray(p_host[rows]),
            "w1t": w1t, "w2t": w2t, "qb1": qb1, "qb2": qb2, "qbc": qbc,
        })

    kw = {}
    if os.environ.get("KERNEL_TRACE_DIR"):
        os.makedirs(os.environ["KERNEL_TRACE_DIR"], exist_ok=True)
        kw["tmpdir"] = os.environ["KERNEL_TRACE_DIR"]
    res = run_bass_kernel_spmd(nc, in_maps, core_ids=list(range(N_CORES)), **kw)
    out = np.concatenate([res.results[i]["o"] for i in range(N_CORES)], axis=0)
    if res.exec_time_ns is not None:
        print(f"HW exec time: {res.exec_time_ns} ns")
    return out.reshape(4, 2048, D).astype(np.float32)


# revision 39
# speedup vs baseline: 1.0215x; 1.0215x over previous
"""Trainium2 Bass kernel for nn_BlockAttnResTransformerBlock (sparse_attention).

Computes, for V = stack([completed_blocks (n=4), partial_block]):
  two inter-block-attention + projection sublayers applied to partial_block.

Everything is row-local over the flattened (b, t) axis (8192 rows), so we
shard 1024 rows per NeuronCore (8 cores, pure SPMD, no collectives).

Math per row r (d = 2048):
  logits_i = (x_i . (q*w_res)) * rsqrt(mean(x_i^2) + eps)   for each block i
  alpha = softmax_i(logits)  ->  h = sum_i alpha_i x_i
  out_sub = (h * rsqrt(mean(h^2)+eps)) @ (proj * w_norm).T
  partial += out_sub      (twice, with the second sublayer's V including the
                           updated partial block)

Key kernel tricks:
  - softmax without max-subtraction (logits are O(+-5)); unnormalized
    exp-weighted sums; 1/Z and rsqrt folded into one per-row scalar.
  - query dots for all blocks computed on the TensorEngine as one M=2
    matmul over host-pre-transposed C (s1 and s2 in a single pass, phase A),
    with the tiny [2 x rows] result transposed back via the xbar.
  - rsqrt via Quake magic-constant + 2 Newton steps (no ACT table loads).
  - residual adds done on the TensorEngine by appending an identity matmul
    (rhs = P/c) to the PSUM accumulation group; the rmsnorm/softmax scalar c
    is applied as the per-partition scale of the PSUM->SBUF output copy, so
    u^T can be transposed the moment the weighted sum finishes.
  - activations/weights in bf16 (fp32 accumulation in PSUM / accum_out).
  - emission is software-pipelined (loads 1 tile ahead, back-half skewed)
    and DMA rings are specialized: SWDGE=plain loads/stores, sync
    HWDGE=xbar transposes only (concurrent copy+transpose on the two HWDGE
    rings hard-hangs the device).
"""

import os
import sys

for _p in ("/opt/trn_rl_repo", "/root/.axon_site/_ro/trn_rl_repo"):
    if os.path.isdir(_p) and _p not in sys.path:
        sys.path.insert(0, _p)

import numpy as np
import ml_dtypes


def _ensure_ntff_hook():
    """Provide antenv.axon_hooks (NTFF profiling) if the image lacks it."""
    try:
        import antenv.axon_hooks  # noqa: F401
        return
    except ImportError:
        pass
    try:
        import types
        import antenv
        if "/root/.axon_site" not in sys.path and os.path.isdir("/root/.axon_site"):
            sys.path.insert(0, "/root/.axon_site")
        from trn_agent_boot.trn_boot import _ntff_profile_via_ctypes
        so = "/opt/axon/libaxon_pjrt.so"
        hook = _ntff_profile_via_ctypes(so) if os.path.exists(so) else None
        mod = types.ModuleType("antenv.axon_hooks")
        state = {"hook": hook}
        mod.get_axon_ntff_profile_hook = lambda: state["hook"]
        mod.set_axon_ntff_profile_hook = lambda h: state.__setitem__("hook", h)
        sys.modules["antenv.axon_hooks"] = mod
        antenv.axon_hooks = mod
    except Exception:
        pass


_ensure_ntff_hook()

import concourse.bass as bass
import concourse.bacc as bacc
import concourse.tile as tile
import concourse.mybir as mybir
from concourse.bass import ts
from concourse.bass_utils import run_bass_kernel_spmd
from concourse.masks import make_identity

BF16 = mybir.dt.bfloat16
F32 = mybir.dt.float32
AF = mybir.ActivationFunctionType
ALU = mybir.AluOpType

N_CORES = 8
N_BLK = 4          # completed blocks
D = 2048
ROWS_TOTAL = 8192  # b*t = 4*2048
R = ROWS_TOTAL // N_CORES   # rows per core
P = 128            # partitions / rows per tile
NT = R // P        # tiles per core (8)
KC = D // P        # contraction chunks (16)
NJ = D // 512      # psum bank chunks (4)
EPS = 1e-6

_CACHED_NC = None


def _fast_rsqrt(nc, statpool, y, x, n, eng=None):
    """y = rsqrt(x) for positive x, [P, n] f32, no ACT tables needed.

    Quake-style magic-constant seed + 2 Newton steps (~5e-6 rel err).
    Runs on `eng` (default gpsimd — tiny ops, keeps DVE free)."""
    eng = eng or nc.gpsimd
    x = x[:, 0:n]
    y = y[:, 0:n]
    iv = statpool.tile([P, n], mybir.dt.int32, tag=f"rsq_i{n}")
    f = statpool.tile([P, n], F32, tag=f"rsq_f{n}")
    t = statpool.tile([P, n], F32, tag=f"rsq_t{n}")
    eng.tensor_copy(out=f, in_=x.bitcast(mybir.dt.int32))  # int -> float
    eng.tensor_scalar(out=f, in0=f, scalar1=-0.5,
                      scalar2=float(0x5F3759DF), op0=ALU.mult, op1=ALU.add)
    eng.tensor_copy(out=iv, in_=f)                         # float -> int
    eng.tensor_copy(out=y.bitcast(mybir.dt.int32), in_=iv)  # raw bits
    for _ in range(2):
        eng.tensor_mul(out=t, in0=y, in1=y)
        eng.tensor_mul(out=t, in0=t, in1=x)
        eng.tensor_scalar(out=t, in0=t, scalar1=-0.5, scalar2=1.5,
                          op0=ALU.mult, op1=ALU.add)
        eng.tensor_mul(out=y, in0=y, in1=t)


def _emit_sublayer(nc, tc, pools, *, c_dram, ct_dram, part_dram, qbc_sb,
                   w_sb, ident, ss_store, s2_store, out_dram, out_dtype,
                   first_phase):
    """Emit one sublayer (8 tiles): part_new = part + proj(attn(V, part)).

    Emission is software-pipelined with a 1-tile skew: tile t's back half
    (x^T transpose + matmuls + writeback) is emitted after tile t+1's front
    half, so the late-dependency x^T DMA never head-of-line blocks the next
    tile's early transposes on the sync HWDGE ring."""
    (cpool, ppool, junkpool, statpool, wsumpool, xpool, xtpool, opool,
     psumpool, ctpool, stgpool, qpsum, wtmppool) = pools
    NB1 = N_BLK + 1
    state = {}

    for t in range(min(1, NT)):
        _emit_loads(nc, pools, state, t, c_dram=c_dram, ct_dram=ct_dram,
                    part_dram=part_dram, first_phase=first_phase)
    for t in range(NT + 1):
        if t + 1 < NT:
            _emit_loads(nc, pools, state, t + 1, c_dram=c_dram,
                        ct_dram=ct_dram, part_dram=part_dram,
                        first_phase=first_phase)
        if t < NT:
            _emit_front(nc, pools, state, t, qbc_sb=qbc_sb,
                        ss_store=ss_store, s2_store=s2_store,
                        first_phase=first_phase)
        if t >= 1:
            _emit_back(nc, pools, state, t - 1, w_sb=w_sb, ident=ident,
                       out_dram=out_dram, out_dtype=out_dtype)


def _emit_loads(nc, pools, state, t, *, c_dram, ct_dram, part_dram,
                first_phase):
    (cpool, ppool, junkpool, statpool, wsumpool, xpool, xtpool, opool,
     psumpool, ctpool, stgpool, qpsum, wtmppool) = pools
    NB1 = N_BLK + 1
    rows = slice(t * P, (t + 1) * P)
    cpt = cpool.tile([P, NB1, D], BF16, tag="c")
    nc.gpsimd.dma_start(out=cpt[:, 0:N_BLK, :], in_=c_dram[rows, :, :])
    nc.gpsimd.dma_start(out=cpt[:, N_BLK, :], in_=part_dram[rows, :])
    if first_phase:
        cT = ctpool.tile([P, N_BLK, KC, P], BF16, tag="cT")
        nc.gpsimd.dma_start(out=cT, in_=ct_dram[t])
    else:
        cT = None
    state[("ld", t)] = (cpt, cT)


def _emit_front(nc, pools, state, t, *, qbc_sb,
                ss_store, s2_store, first_phase):
    (cpool, ppool, junkpool, statpool, wsumpool, xpool, xtpool, opool,
     psumpool, ctpool, stgpool, qpsum, wtmppool) = pools
    NB1 = N_BLK + 1
    if True:
        rows = slice(t * P, (t + 1) * P)
        cpt, cT = state.pop(("ld", t))
        ct = cpt  # [:, i, :] views
        pt = cpt[:, N_BLK, :]

        sps = qpsum.tile([2, NB1 * P], F32, tag="sps")

        # ---- q-dots on the TensorEngine ----------------------------------
        # phase A: one M=2 pass over transposed [C..., P] computes s1 AND s2
        # for all 5 blocks; s2 of the completed blocks is stashed for B.
        # phase B: only the updated partial block needs a fresh dot.
        if first_phase:
            # pre-transposed C came straight from DRAM (host layout prep);
            # only the partial block needs the xbar transpose on device
            ptT = xtpool.tile([P, KC, P], BF16, tag="xt")
            nc.sync.dma_start_transpose(out=ptT, in_=pt)
            for c in range(KC):
                nc.tensor.matmul(sps[0:2, 0:N_BLK * P], lhsT=qbc_sb[:, c, :],
                                 rhs=cT[:, :, c, :], start=(c == 0),
                                 stop=(c == KC - 1))
            for c in range(KC):
                nc.tensor.matmul(sps[0:2, N_BLK * P:NB1 * P],
                                 lhsT=qbc_sb[:, c, :],
                                 rhs=ptT[:, c, :], start=(c == 0),
                                 stop=(c == KC - 1))
            st_stage = stgpool.tile([16, NB1 * P], BF16, tag="st_stage")
            nc.vector.tensor_copy(out=st_stage[0:2, :],
                                  in_=sps[0:2, 0:NB1 * P])
            sT = statpool.tile([P, NB1, 16], BF16, tag="sT")
            nc.sync.dma_start_transpose(out=sT, in_=st_stage[:, :])
            nc.vector.tensor_copy(out=s2_store[:, t * N_BLK:(t + 1) * N_BLK],
                                  in_=sT[:, 0:N_BLK, 1])
        else:
            ptT = xtpool.tile([P, KC, P], BF16, tag="xt")
            nc.sync.dma_start_transpose(out=ptT, in_=pt)
            for c in range(KC):
                nc.tensor.matmul(sps[0:2, 0:P], lhsT=qbc_sb[:, c, :],
                                 rhs=ptT[:, c, :], start=(c == 0),
                                 stop=(c == KC - 1))
            st_stage = stgpool.tile([16, NB1 * P], BF16, tag="st_stage")
            nc.vector.tensor_copy(out=st_stage[0:2, 0:P], in_=sps[0:2, 0:P])
            sT = statpool.tile([P, 16], BF16, tag="sTp")
            nc.sync.dma_start_transpose(out=sT, in_=st_stage[:, 0:P])

        # ---- per-row stats ----------------------------------------------
        # ss (sum of squares) for the 4 completed blocks + partial
        ssall = statpool.tile([P, NB1], F32, tag="ssall")
        if first_phase:
            for i in range(N_BLK):
                junk = junkpool.tile([P, D], BF16, tag="junk")
                nc.scalar.activation(out=junk, in_=ct[:, i, :], func=AF.Square,
                                     accum_out=ssall[:, i:i + 1])
            # stash for phase B (C doesn't change)
            nc.vector.tensor_copy(out=ss_store[:, t * N_BLK:(t + 1) * N_BLK],
                                  in_=ssall[:, 0:N_BLK])
        else:
            nc.vector.tensor_copy(out=ssall[:, 0:N_BLK],
                                  in_=ss_store[:, t * N_BLK:(t + 1) * N_BLK])
        junk = junkpool.tile([P, D], BF16, tag="junk")
        nc.scalar.activation(out=junk, in_=pt, func=AF.Square,
                             accum_out=ssall[:, N_BLK:NB1])

        # logits l = s * rsqrt(ss/D + eps)
        m = statpool.tile([P, NB1], F32, tag="m")
        nc.vector.tensor_scalar(out=m, in0=ssall, scalar1=1.0 / D, scalar2=EPS,
                                op0=ALU.mult, op1=ALU.add)
        rms = statpool.tile([P, NB1], F32, tag="rms")
        _fast_rsqrt(nc, statpool, rms, m, NB1)
        lg = statpool.tile([P, NB1], F32, tag="lg")
        if first_phase:
            nc.vector.tensor_mul(out=lg, in0=sT[:, :, 0], in1=rms)
        else:
            nc.vector.tensor_mul(out=lg[:, 0:N_BLK],
                                 in0=s2_store[:, t * N_BLK:(t + 1) * N_BLK],
                                 in1=rms[:, 0:N_BLK])
            nc.vector.tensor_mul(out=lg[:, N_BLK:NB1], in0=sT[:, 1:2],
                                 in1=rms[:, N_BLK:NB1])
        ew = statpool.tile([P, N_BLK + 1], F32, tag="ew")
        nc.scalar.activation(out=ew, in_=lg, func=AF.Exp)
        zr = statpool.tile([P, 2], F32, tag="zr")
        nc.vector.reduce_sum(out=zr[:, 0:1], in_=ew, axis=mybir.AxisListType.X)
        nc.vector.reciprocal(out=zr[:, 1:2], in_=zr[:, 0:1])  # r = 1/Z

        # ---- unnormalized weighted sum u = sum_i E_i * V_i ---------------
        # tensor_scalar (bf16 4x) + tensor_add (bf16 2x) per block
        w_acc = wsumpool.tile([P, D], BF16, tag="wsum")
        nc.vector.tensor_scalar(out=w_acc, in0=ct[:, 0, :],
                                scalar1=ew[:, 0:1], scalar2=None, op0=ALU.mult)
        for i in range(1, N_BLK + 1):
            src = pt if i == N_BLK else ct[:, i, :]
            tmp = wtmppool.tile([P, D], BF16, tag="wtmp")
            nc.vector.tensor_scalar(out=tmp, in0=src, scalar1=ew[:, i:i + 1],
                                    scalar2=None, op0=ALU.mult)
            w_next = wsumpool.tile([P, D], BF16, tag="wsum")
            nc.vector.tensor_add(out=w_next, in0=tmp, in1=w_acc)
            w_acc = w_next
        u = w_acc

        # ---- norm scalar c = r * rsqrt(r^2*ssu/D + eps) ------------------
        ssu = statpool.tile([P, 4], F32, tag="ssu")
        junk = junkpool.tile([P, D], BF16, tag="junk")
        nc.scalar.activation(out=junk, in_=u, func=AF.Square,
                             accum_out=ssu[:, 0:1])
        nc.vector.tensor_mul(out=ssu[:, 1:2], in0=zr[:, 1:2], in1=zr[:, 1:2])
        nc.vector.tensor_scalar(out=ssu[:, 2:3], in0=ssu[:, 0:1],
                                scalar1=ssu[:, 1:2], scalar2=1.0 / D,
                                op0=ALU.mult, op1=ALU.mult)
        nc.vector.tensor_scalar(out=ssu[:, 2:3], in0=ssu[:, 2:3], scalar1=EPS,
                                scalar2=None, op0=ALU.add)
        rsu = statpool.tile([P, 1], F32, tag="rsu")
        _fast_rsqrt(nc, statpool, rsu, ssu[:, 2:3], 1)
        nc.vector.tensor_mul(out=ssu[:, 3:4], in0=rsu, in1=zr[:, 1:2])

        # c is applied at the output copy instead of scaling u (so u^T can
        # be transposed as soon as the weighted sum finishes); the residual
        # is added as P/c via the identity matmul.
        rc = statpool.tile([P, 1], F32, tag="rc")
        nc.vector.reciprocal(out=rc, in_=ssu[:, 3:4])
        ptc = xpool.tile([P, D], BF16, tag="x")
        nc.vector.tensor_scalar(out=ptc, in0=pt, scalar1=rc,
                                scalar2=None, op0=ALU.mult)
        state[t] = (pt, ptc, u, ssu)


def _emit_back(nc, pools, state, t, *, w_sb, ident, out_dram, out_dtype):
    (cpool, ppool, junkpool, statpool, wsumpool, xpool, xtpool, opool,
     psumpool, ctpool, stgpool, qpsum, wtmppool) = pools
    pt, ptc, u, ssu = state.pop(t)
    rows = slice(t * P, (t + 1) * P)

    ut = xtpool.tile([P, KC, P], BF16, tag="xt")
    nc.sync.dma_start_transpose(out=ut, in_=u)

    # ---- projection matmul (two 2-bank halves) + residual add ------------
    po = opool.tile([P, D], out_dtype, tag="po")
    for h in range(2):
        psh = psumpool.tile([P, 1024], F32, tag="mm")
        for k in range(KC):
            for j in range(2):
                nc.tensor.matmul(psh[:, ts(j, 512)], lhsT=ut[:, k, :],
                                 rhs=w_sb[k][:, ts(h * 2 + j, 512)],
                                 start=(k == 0), stop=False)
        for j in range(2):
            nc.tensor.matmul(psh[:, ts(j, 512)], lhsT=ident,
                             rhs=ptc[:, ts(h * 2 + j, 512)], start=False,
                             stop=True)
        # out = c * (u @ W + P/c)  -- c applied as the copy's scale
        nc.scalar.activation(out=po[:, ts(h, 1024)], in_=psh, func=AF.Copy,
                             scale=ssu[:, 3:4])
    nc.gpsimd.dma_start(out=out_dram[rows, :], in_=po)


def _build_nc():
    nc = bacc.Bacc("TRN2", target_bir_lowering=False, debug=False,
                   num_devices=N_CORES)

    c_in = nc.dram_tensor("c", [R, N_BLK, D], BF16, kind="ExternalInput")
    ct_in = nc.dram_tensor("ctr", [NT, P, N_BLK, KC, P], BF16,
                           kind="ExternalInput")
    p_in = nc.dram_tensor("p", [R, D], BF16, kind="ExternalInput")
    w1_in = nc.dram_tensor("w1t", [D, D], BF16, kind="ExternalInput")
    w2_in = nc.dram_tensor("w2t", [D, D], BF16, kind="ExternalInput")
    qbc_in = nc.dram_tensor("qbc", [P, KC, 2], BF16, kind="ExternalInput")
    o_out = nc.dram_tensor("o", [R, D], F32, kind="ExternalOutput")
    p1_mid = nc.dram_tensor("p1mid", [R, D], BF16)  # internal scratch

    with tile.TileContext(nc) as tc:
        with (
            tc.tile_pool(name="singles", bufs=1) as singles,
            tc.tile_pool(name="weights", bufs=1) as wpool,
            tc.tile_pool(name="cpool", bufs=2) as cpool,
            tc.tile_pool(name="ppool", bufs=2) as ppool,
            tc.tile_pool(name="junk", bufs=1) as junkpool,
            tc.tile_pool(name="stat", bufs=5) as statpool,
            tc.tile_pool(name="wsum", bufs=3) as wsumpool,
            tc.tile_pool(name="wtmp", bufs=2) as wtmppool,
            tc.tile_pool(name="xpool", bufs=2) as xpool,
            tc.tile_pool(name="xtpool", bufs=2) as xtpool,
            tc.tile_pool(name="opool", bufs=2) as opool,
            tc.tile_pool(name="psum", bufs=3, space="PSUM") as psumpool,
            tc.tile_pool(name="qpsum", bufs=1, space="PSUM") as qpsum,
            tc.tile_pool(name="ctpool", bufs=2) as ctpool,
            tc.tile_pool(name="stgpool", bufs=2) as stgpool,
        ):
            ident = singles.tile([P, P], BF16)
            make_identity(nc, ident)
            qbc = singles.tile([P, KC, 2], BF16)
            nc.sync.dma_start(out=qbc, in_=qbc_in[:, :, :])
            ss_store = singles.tile([P, NT * N_BLK], F32)
            s2_store = singles.tile([P, NT * N_BLK], BF16)

            w1_view = w1_in.ap().rearrange("(c q) j -> c q j", q=P)
            w2_view = w2_in.ap().rearrange("(c q) j -> c q j", q=P)

            pools = (cpool, ppool, junkpool, statpool, wsumpool, xpool,
                     xtpool, opool, psumpool, ctpool, stgpool, qpsum,
                     wtmppool)

            # phase A: sublayer 1 (attn): p1mid = p + attn_out
            w1_sb = []
            for k in range(KC):
                wk = wpool.tile([P, D], BF16, tag=f"w{k}")
                nc.sync.dma_start(out=wk, in_=w1_view[k])
                w1_sb.append(wk)
            _emit_sublayer(nc, tc, pools, c_dram=c_in.ap(),
                           ct_dram=ct_in.ap(), part_dram=p_in.ap(),
                           qbc_sb=qbc, w_sb=w1_sb, ident=ident,
                           ss_store=ss_store, s2_store=s2_store,
                           out_dram=p1_mid.ap(),
                           out_dtype=BF16, first_phase=True)

            # phase B: sublayer 2 (mlp): o = p1mid + mlp_out
            w2_sb = []
            for k in range(KC):
                wk = wpool.tile([P, D], BF16, tag=f"w{k}")
                nc.sync.dma_start(out=wk, in_=w2_view[k])
                w2_sb.append(wk)
            _emit_sublayer(nc, tc, pools, c_dram=c_in.ap(),
                           ct_dram=ct_in.ap(), part_dram=p1_mid.ap(),
                           qbc_sb=qbc, w_sb=w2_sb, ident=ident,
                           ss_store=ss_store, s2_store=s2_store,
                           out_dram=o_out.ap(),
                           out_dtype=F32, first_phase=False)

    nc.compile()
    return nc


def _get_nc():
    global _CACHED_NC
    if _CACHED_NC is None:
        _CACHED_NC = _build_nc()
    return _CACHED_NC


def kernel(completed_blocks, partial_block, attn_norm_w, attn_proj,
           mlp_norm_w, mlp_proj, attn_res_query, attn_res_norm_w,
           mlp_res_query, mlp_res_norm_w, layer_in_block=1, **_ignored):
    bf16 = ml_dtypes.bfloat16
    cb = np.asarray(completed_blocks, np.float32)
    pb = np.asarray(partial_block, np.float32)

    # [n, b, t, d] -> [rows, n, d]
    c_host = np.ascontiguousarray(
        np.moveaxis(cb.reshape(N_BLK, ROWS_TOTAL, D), 0, 1)).astype(bf16)
    # pre-transposed per-core layout for the PE q-dots:
    # ctr[t, p, i, c, r] = C[t*128+r, i, c*128+p]
    ctr_host = np.ascontiguousarray(
        c_host.reshape(ROWS_TOTAL // P, P, N_BLK, KC, P)
        .transpose(0, 4, 2, 3, 1))
    p_host = pb.reshape(ROWS_TOTAL, D).astype(bf16)

    # fold the post-attention norm gain into the projection, transpose to [k, j]
    w1t = np.ascontiguousarray(
        (np.asarray(attn_proj, np.float32)
         * np.asarray(attn_norm_w, np.float32)[None, :]).T).astype(bf16)
    w2t = np.ascontiguousarray(
        (np.asarray(mlp_proj, np.float32)
         * np.asarray(mlp_norm_w, np.float32)[None, :]).T).astype(bf16)

    # fold the K-norm gain into the query; pre-broadcast across partitions
    qb1 = np.broadcast_to(
        (np.asarray(attn_res_query, np.float32)
         * np.asarray(attn_res_norm_w, np.float32)).astype(bf16), (P, D)).copy()
    qb2 = np.broadcast_to(
        (np.asarray(mlp_res_query, np.float32)
         * np.asarray(mlp_res_norm_w, np.float32)).astype(bf16), (P, D)).copy()
    # chunked lhsT for the PE q-dots: qbc[p, c, i] = q_i[c*128 + p]
    qbc = np.ascontiguousarray(
        np.stack([qb1[0].reshape(KC, P).T, qb2[0].reshape(KC, P).T],
                 axis=-1))  # [P, KC, 2] bf16

    nc = _get_nc()
    in_maps = []
    for i in range(N_CORES):
        rows = slice(i * R, (i + 1) * R)
        in_maps.append({
            "c": np.ascontiguousarray(c_host[rows]),
            "ctr": np.ascontiguousarray(ctr_host[i * NT:(i + 1) * NT]),
            "p": np.ascontiguousarray(p_host[rows]),
            "w1t": w1t, "w2t": w2t, "qbc": qbc,
        })

    kw = {}
    if os.environ.get("KERNEL_TRACE_DIR"):
        os.makedirs(os.environ["KERNEL_TRACE_DIR"], exist_ok=True)
        kw["tmpdir"] = os.environ["KERNEL_TRACE_DIR"]
    res = run_bass_kernel_spmd(nc, in_maps, core_ids=list(range(N_CORES)), **kw)
    out = np.concatenate([res.results[i]["o"] for i in range(N_CORES)], axis=0)
    if res.exec_time_ns is not None:
        print(f"HW exec time: {res.exec_time_ns} ns")
    return out.reshape(4, 2048, D).astype(np.float32)
